# revision 3
# baseline (speedup 1.0000x reference)
"""Noisy top-k (k=2) router for Trainium2, data-parallel over 8 NeuronCores.

Math: for each row of noisy = logits + noise, the top-2 softmax weights are
    w1 = sigmoid(v1 - v2),  w2 = sigmoid(v2 - v1)   (v1 >= v2 top-2 values)
and sigmoid(2*x - (v1 + v2)) equals w1 at x = v1 and w2 at x = v2. So the
scattered output is
    out[x] = (x >= v2) * sigmoid(2*x - (v1 + v2))
which needs no indices at all. The (x >= v2) compare runs on the exact fp32
input values, so selection is bit-exact as long as no row has v2 == v3 ties
(holds for these inputs).

vs the 101us baseline:
  - bf16 mask/sigmoid/output: the final multiply runs in DVE 2x_1p mode and
    the output store moves half the bytes (rel err ~1.7e-3, gate 2e-2).
  - noise adds: chunk 0 on DVE (fast ramp), chunks 1-3 via SWDGE CCE
    accumulate spread across 4 SWDGE queues (the CCE add runs ~0.42x of
    plain DMA rate, so one queue cannot keep up with compute).
  - t=0 warm-up: dummy 256B accums on all 4 SWDGE queues (the first SWDGE
    accum on a queue pays a ~20us spin-up) + dummy sigmoid to preload the
    ACT function table (1.3us) during the ramp.
  - DVE emission order software-pipelines mult(h-1) behind mask(h) so DVE
    never waits on ACT's sigmoid of the current half.
Engines: DVE = Max8 + mask + mult (+chunk-0 adds); ACT = per-row sigmoid
(bias = -(v1+v2), scale = 2); Pool = SWDGE accum issue; Sync = load/store
issue. This walrus codegen allows only ONE sync-wait per instruction; the
_legalize_waits post-pass splits any excess into standalone EventSemaphore
instructions (which hold two).
"""

import time

import numpy as np

import concourse.bass as bass
import concourse.mybir as mybir
from concourse.tile import TileContext
from concourse.bass_utils import run_bass_kernel_spmd

B = 262144
E = 64
N_CORES = 8
B_CORE = B // N_CORES  # 32768 rows per core

P = 128  # SBUF partitions
NC_ = 4  # DMA chunks per core (16KB contiguous per partition each)
NH = 2  # compute halves per chunk
RH = 32  # rows per partition per half
# rows per partition total = NC_*NH*RH = 256; row = ((p*NC_+c)*NH+h)*RH+r

_CACHE = {}

# test.py introspection: BassKernelResults of the most recent run
LAST_RESULT = None


def _legalize_waits(nc: "bass.Bass") -> None:
    """This walrus codegen accepts at most ONE sync-wait per instruction
    (two on EventSemaphore). Tile's wait assigner can emit more; split the
    excess into standalone EventSemaphore waits placed immediately before
    the instruction on the same engine (identical semantics: the engine
    blocks there instead)."""
    n = 0
    for fnb in nc.m.functions[0].blocks:
        out = []
        for inst in fnb.instructions:
            si = inst.sync_info
            cap = 2 if isinstance(inst, mybir.InstEventSemaphore) else 1
            if si is not None and len(si.on_wait) > cap:
                waits = list(si.on_wait)
                extra, keep = waits[:-cap], waits[-cap:]
                for c in range(0, len(extra), 2):
                    n += 1
                    out.append(
                        mybir.InstEventSemaphore(
                            name=f"EVW-{n}",
                            engine=inst.engine,
                            sync_info=mybir.SyncInfo(
                                on_wait=extra[c : c + 2], on_update=[]
                            ),
                        )
                    )
                inst.sync_info = mybir.SyncInfo(
                    on_wait=keep, on_update=list(si.on_update)
                )
            out.append(inst)
        fnb.instructions = out


def _build_nc() -> bass.Bass:
    nc = bass.Bass()
    f32 = mybir.dt.float32
    bf16 = mybir.dt.bfloat16

    lg = nc.dram_tensor("logits", [B_CORE, E], f32, kind="ExternalInput")
    nz = nc.dram_tensor("noise", [B_CORE, E], f32, kind="ExternalInput")
    out = nc.dram_tensor("out", [B_CORE, E], bf16, kind="ExternalOutput")

    # partition-major layout: partition p owns 256 contiguous DRAM rows,
    # split into NC_ chunks of NH*RH rows -> 16KB contiguous per partition
    # per chunk (8KB per half)
    lgv = lg[:].rearrange("(p c h r) e -> c p h r e", p=P, c=NC_, h=NH)
    nzv = nz[:].rearrange("(p c h r) e -> c p h r e", p=P, c=NC_, h=NH)
    outv = out[:].rearrange("(p c h r) e -> c p h r e", p=P, c=NC_, h=NH)

    with TileContext(nc) as tc:
        with (
            tc.tile_pool(name="chunk", bufs=4) as chunk_pool,
            tc.tile_pool(name="sub", bufs=3) as sub_pool,
        ):
            noisy_tiles = {}
            pending = []  # (mask, sig, c, h) awaiting mult+store

            # --- t=0 warm-ups ---
            # dummy 256B accum: the first SWDGE CCE accum pays a ~9us
            # spin-up; eat it during the ramp. NOTHING depends on it.
            warm = chunk_pool.tile([P, 4, E], f32, tag="warm", bufs=1)
            nc.gpsimd.memset(warm, 0.0)
            nc.gpsimd.dma_start(
                out=warm[:, 0],
                in_=nzv[0][:, 0, 0, :],
                accum_op=mybir.AluOpType.add,
            )
            # dummy sigmoid on a memset-only tile: preload the ACT function
            # table (1.3us) during the ramp without any accum dependency
            wsrc = sub_pool.tile([P, 8], f32, tag="wsrc", bufs=1)
            nc.gpsimd.memset(wsrc, 0.0)
            wsig = sub_pool.tile([P, 8], bf16, tag="wsig", bufs=1)
            nc.scalar.activation(
                out=wsig,
                in_=wsrc,
                func=mybir.ActivationFunctionType.Sigmoid,
                scale=2.0,
            )

            def issue_load(c):
                noisy = chunk_pool.tile([P, NH, RH, E], f32, tag="noisy")
                noisy_tiles[c] = noisy
                # 8KB half-loads: each accum(c,h) can start as soon as its
                # own half landed
                for h in range(NH):
                    nc.sync.dma_start(out=noisy[:, h], in_=lgv[c][:, h])

            HB = 50.0

            def compute(c, h, full_sigmoid=False, rows=(0, RH)):
                r0, r1 = rows
                RG = r1 - r0
                noisy = noisy_tiles[c]
                x = noisy[:, h, r0:r1]  # [P, RG, E]

                # top-8 per row; slots 0 (v1) and 1 (v2)
                v8 = sub_pool.tile([P, RG, 8], f32, tag=f"v8{RG}")
                for r in range(RG):
                    nc.vector.max(out=v8[:, r, :], in_=x[:, r, :])

                # negs = -(v1 + v2) = (v1 * -1) - v2
                negs = sub_pool.tile([P, RG], f32, tag=f"negs{RG}")
                nc.vector.scalar_tensor_tensor(
                    out=negs,
                    in0=v8[:, :, 0],
                    scalar=-1.0,
                    in1=v8[:, :, 1],
                    op0=mybir.AluOpType.mult,
                    op1=mybir.AluOpType.subtract,
                )

                if full_sigmoid:
                    # tail variant: one full-tile sigmoid instead of 32
                    # per-row ones (7.7us -> 1.9us of ACT in the tail).
                    # z = x - (s/2 + HB) + HB*m; sigmoid(2z) = masked output
                    # directly (non-top-2 args <= -60 underflow to 0).
                    cr = sub_pool.tile([P, RG], f32, tag=f"cr{RG}")
                    nc.vector.tensor_scalar(
                        out=cr,
                        in0=negs,
                        scalar1=-0.5,
                        scalar2=HB,
                        op0=mybir.AluOpType.mult,
                        op1=mybir.AluOpType.add,
                    )
                    mask = sub_pool.tile([P, RG, E], f32, tag=f"maskf{RG}")
                    nc.vector.tensor_tensor(
                        out=mask,
                        in0=x,
                        in1=v8[:, :, 1].to_broadcast([P, RG, E]),
                        op=mybir.AluOpType.is_ge,
                    )
                    yp = sub_pool.tile([P, RG, E], f32, tag=f"yp{RG}")
                    nc.vector.tensor_tensor(
                        out=yp,
                        in0=x,
                        in1=cr.to_broadcast([P, RG, E]),
                        op=mybir.AluOpType.subtract,
                    )
                    z = sub_pool.tile([P, RG, E], f32, tag=f"z{RG}")
                    nc.vector.scalar_tensor_tensor(
                        out=z,
                        in0=mask,
                        scalar=HB,
                        in1=yp,
                        op0=mybir.AluOpType.mult,
                        op1=mybir.AluOpType.add,
                    )
                    o = sub_pool.tile([P, RG, E], bf16, tag=f"o{RG}")
                    nc.scalar.activation(
                        out=o,
                        in_=z,
                        func=mybir.ActivationFunctionType.Sigmoid,
                        scale=2.0,
                    )
                    nc.sync.dma_start(out=outv[c][:, h, r0:r1], in_=o)
                    return

                # sig = sigmoid(2*noisy - (v1+v2)) per row (bias is per-row)
                sig = sub_pool.tile([P, RG, E], bf16, tag=f"sig{RG}")
                for r in range(RG):
                    nc.scalar.activation(
                        out=sig[:, r, :],
                        in_=x[:, r, :],
                        func=mybir.ActivationFunctionType.Sigmoid,
                        bias=negs[:, r : r + 1],
                        scale=2.0,
                    )

                # mask = (noisy >= v2), exact fp32 compare -> bf16 {0,1}
                mask = sub_pool.tile([P, RG, E], bf16, tag=f"mask{RG}")
                nc.vector.tensor_tensor(
                    out=mask,
                    in0=x,
                    in1=v8[:, :, 1].to_broadcast([P, RG, E]),
                    op=mybir.AluOpType.is_ge,
                )

                # mult(h-1) on DVE now, after mask(h): sigmoid(h-1) finished
                # long ago so DVE never stalls on ACT
                while pending:
                    pm, ps, c_, h_, rr = pending.pop(0)
                    RG_ = rr[1] - rr[0]
                    o = sub_pool.tile([P, RG_, E], bf16, tag=f"o{RG_}")
                    nc.vector.tensor_tensor(
                        out=o, in0=pm, in1=ps, op=mybir.AluOpType.mult
                    )
                    nc.sync.dma_start(
                        out=outv[c_][:, h_, rr[0] : rr[1]], in_=o
                    )
                pending.append((mask, sig, c, h, rows))

            # ramp: chunk-0 lg and nz loads interleaved FIRST on qSP (queue
            # FIFO order -> first compute data lands earliest; a separate
            # queue would fair-share DMA engines with the chunk load flood
            # and finish LATER). h0 in 4KB quarters so the first 16-row
            # compute group starts ~3us earlier. Remaining chunk loads after.
            RQ = RH // 2
            nzt = chunk_pool.tile([P, NH, RH, E], f32, tag="nzt", bufs=1)
            noisy0 = chunk_pool.tile([P, NH, RH, E], f32, tag="noisy")
            noisy_tiles[0] = noisy0
            for q in range(2):
                sl = slice(q * RQ, (q + 1) * RQ)
                nc.sync.dma_start(out=noisy0[:, 0, sl], in_=lgv[0][:, 0, sl])
                nc.sync.dma_start(out=nzt[:, 0, sl], in_=nzv[0][:, 0, sl])
            nc.sync.dma_start(out=noisy0[:, 1], in_=lgv[0][:, 1])
            nc.sync.dma_start(out=nzt[:, 1], in_=nzv[0][:, 1])
            # chunk 1 also takes the plain-load + DVE-add path: its loads
            # queue behind the ramp on qSP and its data is needed early —
            # the serial SWDGE accum stream cannot produce it in time
            nzt1 = chunk_pool.tile([P, NH, RH, E], f32, tag="nzt1", bufs=1)
            noisy1 = chunk_pool.tile([P, NH, RH, E], f32, tag="noisy")
            noisy_tiles[1] = noisy1
            for h in range(NH):
                nc.sync.dma_start(out=noisy1[:, h], in_=lgv[1][:, h])
                nc.sync.dma_start(out=nzt1[:, h], in_=nzv[1][:, h])
            for c in range(2, NC_):
                issue_load(c)

            # chunks 2-3: SWDGE CCE accumulate (waits on the matching lg
            # load resolve in issue order on the Pool engine)
            for c in range(2, NC_):
                for h in range(NH):
                    nc.gpsimd.dma_start(
                        out=noisy_tiles[c][:, h],
                        in_=nzv[c][:, h],
                        accum_op=mybir.AluOpType.add,
                    )

            # chunk 0 h0: DVE add + compute per 16-row quarter so the first
            # sigmoid batch starts as early as possible
            for q in range(2):
                sl = slice(q * RQ, (q + 1) * RQ)
                nc.vector.tensor_tensor(
                    out=noisy0[:, 0, sl],
                    in0=noisy0[:, 0, sl],
                    in1=nzt[:, 0, sl],
                    op=mybir.AluOpType.add,
                )
                compute(0, 0, rows=(q * RQ, (q + 1) * RQ))
            nc.vector.tensor_tensor(
                out=noisy0[:, 1],
                in0=noisy0[:, 1],
                in1=nzt[:, 1],
                op=mybir.AluOpType.add,
            )
            compute(0, 1)

            for c in range(1, NC_):
                for h in range(NH):
                    if c == 1:
                        nc.vector.tensor_tensor(
                            out=noisy1[:, h],
                            in0=noisy1[:, h],
                            in1=nzt1[:, h],
                            op=mybir.AluOpType.add,
                        )
                    if c == NC_ - 1 and h == NH - 1:
                        # tail: last per-row-sigmoid batch halved (16 rows),
                        # then the very end as one full-tile sigmoid
                        compute(c, h, rows=(0, RQ))
                        compute(c, h, full_sigmoid=True, rows=(RQ, RH))
                    else:
                        compute(c, h)
            # final mult+store
            while pending:
                pm, ps, c_, h_, rr = pending.pop(0)
                RG_ = rr[1] - rr[0]
                o = sub_pool.tile([P, RG_, E], bf16, tag=f"o{RG_}")
                nc.vector.tensor_tensor(
                    out=o, in0=pm, in1=ps, op=mybir.AluOpType.mult
                )
                nc.sync.dma_start(out=outv[c_][:, h_, rr[0] : rr[1]], in_=o)

    _legalize_waits(nc)
    return nc


def _get_nc() -> bass.Bass:
    if "nc" not in _CACHE:
        _CACHE["nc"] = _build_nc()
    return _CACHE["nc"]


def kernel(logits: np.ndarray, noise: np.ndarray) -> np.ndarray:
    global LAST_RESULT
    logits = np.ascontiguousarray(np.asarray(logits), dtype=np.float32)
    noise = np.ascontiguousarray(np.asarray(noise), dtype=np.float32)
    assert logits.shape == (B, E) and noise.shape == (B, E)

    lg_shards = np.split(logits, N_CORES, axis=0)
    nz_shards = np.split(noise, N_CORES, axis=0)
    in_maps = [
        {"logits": lg_shards[i], "noise": nz_shards[i]} for i in range(N_CORES)
    ]

    try:
        res = run_bass_kernel_spmd(
            _get_nc(), in_maps, core_ids=list(range(N_CORES))
        )
    except Exception:
        # transient NRT device errors have been observed right after a
        # crashed run; one retry clears them
        time.sleep(5)
        res = run_bass_kernel_spmd(
            _get_nc(), in_maps, core_ids=list(range(N_CORES))
        )
    LAST_RESULT = res
    return np.concatenate(
        [r["out"].astype(np.float32) for r in res.results], axis=0
    )
